# revision 35
# baseline (speedup 1.0000x reference)
"""CostVolume2D Trainium2 Bass kernel (v4: no scratch roundtrip).

cost[n,d,h,w] = mean_c l[n,c,h,w] * r[n,c,h,w-d]  (0 for w < d)
N=8, C=32, H=256, W=512, D=64.  Data-parallel over batch: core i handles n=i.

vs v2 (519 us): the DRAM scratch shear + gather + DVE block-transpose +
output store (50 MB of round-trip traffic, ~130K tiny 192B descriptors,
205 us of DVE transpose time) are gone.  The device stores the *band*
tiles in a windowed SBUF layout and the host extracts the diagonals with
a numpy as_strided view.  Per-core traffic: 16.8 MB in + 25.2 MB out,
80 DMAs, every descriptor >= 2 KB.

v3 lesson baked in here: engine copies with a strided dst run at ~5-6.7
cycles/elem (the cost model only counts ap size).  So the band layout is
[wp][jb(12)][rr(32)][ji(32)]: the psum->SBUF copy writes 64 B contiguous
runs (near-full rate), and the (c, k) store windows are whole jb blocks
-> 2 KB descriptors.

Device pipeline per 32-row block b (loads prefetched 2 blocks ahead on
sync, placed before the stores in stream order):
  - 2 half input DMAs: host-prebuilt [128, 8192] image (no l pad:
    the (wp1,k1) matmul is trimmed to 128 moving cols since w >= 512 is
    host-discarded), partition 32*jj+c holds row h = 32b + 4g + jj.
  - per row-quad, per wp: 8 matmuls into a 4-bank psum tile [128, 2048]
    (row q at 512-col offset; cols 192k+t hold M[v,w] with
    v = 256wp+128k+p, w = 256wp+128k+t; valid band t in [p, p+64)).
  - 1 copy per psum tile (scale 1/C, f32->bf16), alternating DVE/Act.
  - per (wp, c): 1 store DMA covering both k windows (band jb in
    [c+6k, +3), i.e. cols j in [192k+32c, +96)), 6 KB descriptors.
Host: as_strided gather j' = p' + d, transpose to [d, h, w], zero-fill
the w < d triangle.
"""

import numpy as np

_CACHE = {}

C, H, W, D = 32, 256, 512, 64
N_CORES = 8
R = 32                    # rows per block
NB = H // R               # 8 blocks
INCOLS = 8192             # [l g0-3 | r g0-3 | l g4-7 | r g4-7], 512 each
JI = 32                   # band inner-run elems (rr-major blocks of ji)
JB = 384 // JI            # 12 j-blocks per wp
BLK = R * JI              # 1024: elems per (partition, jb)
PIECE = 32 * 3 * BLK      # 98304 elems per (b, wp, c, k) piece


def _lcol(g):
    return (g // 4) * 4096 + (g % 4) * 512


def _rcol(g):
    return (g // 4) * 4096 + 2048 + (g % 4) * 512


def _build():
    import concourse.tile as tile
    from concourse import bacc, mybir
    from concourse.ap import AP

    f32 = mybir.dt.float32
    bf16 = mybir.dt.bfloat16

    nc = bacc.Bacc("TRN2", target_bir_lowering=False, debug=False)
    x_d = nc.dram_tensor("x", [NB, 128, INCOLS], bf16, kind="ExternalInput")
    o_d = nc.dram_tensor("o", [1, NB * 2 * 4 * 2 * PIECE], bf16,
                         kind="ExternalOutput")

    with tile.TileContext(nc) as tc:
        with (
            tc.tile_pool(name="io", bufs=2) as io_pool,
            tc.tile_pool(name="band", bufs=2) as band_pool,
            tc.tile_pool(name="psum", bufs=2, space="PSUM") as psum_pool,
        ):
            ci = 0
            in_tiles = {}

            def load(bb):
                t = io_pool.tile([128, INCOLS], bf16, tag="in")
                # two half-loads (g 0-3 then g 4-7) so the first row-quads
                # can start before the whole block arrives
                for hh in range(2):
                    nc.sync.dma_start(
                        t[:, hh * 4096:(hh + 1) * 4096],
                        AP(x_d.ap().tensor,
                           bb * 128 * INCOLS + hh * 4096,
                           [[INCOLS, 128], [1, 4096]]))
                in_tiles[bb] = t

            load(0)
            load(1)
            for b in range(NB):
                it = in_tiles.pop(b)

                # band layout per partition:
                #   wp*(12*1024) + jb*1024 + rr*32 + ji   (j = 32*jb + ji)
                band = band_pool.tile([128, 2 * 12 * BLK], bf16, tag="band")
                bandv = band[:].rearrange(
                    "p (wp jb rr ji) -> p wp rr jb ji",
                    wp=2, jb=JB, rr=R, ji=JI)
                for rq in range(R // 4):
                    for wp in range(2):
                        ps = psum_pool.tile([128, 2048], f32, tag="ps")
                        # k-outer so consecutive matmuls hit different
                        # psum banks (q), avoiding same-bank turnaround
                        for k in range(2):
                            for q in range(4):
                                rr = rq * 4 + q
                                g = rr // 4
                                j = rr % 4
                                lrow = it[32 * j:32 * j + 32,
                                          _lcol(g):_lcol(g) + W]
                                rrow = it[32 * j:32 * j + 32,
                                          _rcol(g):_rcol(g) + W]
                                wb = (2 * wp + k) * 128
                                # (wp1,k1): cols w >= 512 are discarded by
                                # the host, so 128 moving cols suffice (the
                                # uncomputed psum tail is stored-garbage in
                                # discarded slots only)
                                mc = 128 if wb == 384 else 192
                                nc.tensor.matmul(
                                    ps[:, 512 * q + 192 * k:
                                       512 * q + 192 * k + mc],
                                    rrow[:, wb:wb + 128],
                                    lrow[:, wb:wb + mc],
                                    start=True, stop=True,
                                    tile_position=(32 * j, 0),
                                )
                        # copy row-quad: src [p, q(4), jb(12), ji(32)] ->
                        # dst [p, rr(4): stride 32, jb: stride 1024, ji: 1]
                        src = ps[:].rearrange(
                            "p (r jb ji) -> p r jb ji",
                            r=4, jb=16, ji=JI)[:, :, 0:JB, :]
                        dst = bandv[:, wp, rq * 4:rq * 4 + 4, :, :]
                        if ci % 2 == 0:
                            nc.vector.tensor_scalar_mul(dst, src, 1.0 / C)
                        else:
                            nc.scalar.mul(dst, src, 1.0 / C)
                        ci += 1

                # prefetch the b+2 input before this block's stores so the
                # load isn't stuck behind store sem-waits in sync's stream
                if b + 2 < NB:
                    load(b + 2)

                # stores: piece (b, wp, c) = [p'(32), k(2), jbw(3), blk],
                # blk = rr*32 + ji; band jb = 6k + c + jbw.  One DMA per
                # (wp, c): both k windows (band k-stride 6*BLK is uniform).
                for wp in range(2):
                    for c in range(4):
                        pb = ((b * 2 + wp) * 4 + c) * 2 * PIECE
                        sv = band[32 * c:32 * c + 32, :].rearrange(
                            "p (wpp kk rest) -> p wpp kk rest",
                            wpp=2, kk=2, rest=6 * BLK)
                        src = sv[:, wp, :, c * BLK:c * BLK + 3 * BLK]
                        nc.sync.dma_start(
                            AP(o_d.ap().tensor, pb,
                               [[2 * 3 * BLK, 32], [3 * BLK, 2], [1, 3 * BLK]]),
                            src)
    nc.compile()
    return nc


def _get_nc():
    if "nc" not in _CACHE:
        _CACHE["nc"] = _build()
    return _CACHE["nc"]


def _in_maps(l_fmap, r_fmap):
    import ml_dtypes

    bf = ml_dtypes.bfloat16
    n = N_CORES
    # x[n, b, 32*jj + c, _lcol(g) + w] = l[n, c, 32b + 4g + jj, w]
    # x[n, b, 32*jj + c, _rcol(g) + w] = r[n, c, 32b + 4g + jj, w]
    xin = np.zeros((n, NB, 128, INCOLS), dtype=bf)
    # axes [n, c, b, g, jj, w] -> [n, b, (jj, c)=128, g, w]
    lv = l_fmap.astype(bf).reshape(n, C, NB, 8, 4, W).transpose(
        0, 2, 4, 1, 3, 5).reshape(n, NB, 128, 8, W)
    rv = r_fmap.astype(bf).reshape(n, C, NB, 8, 4, W).transpose(
        0, 2, 4, 1, 3, 5).reshape(n, NB, 128, 8, W)
    xv = xin.reshape(n, NB, 128, 2, 2, 4, W)  # [.., half, l/r, g%4, w]
    xv[:, :, :, 0, 0] = lv[:, :, :, 0:4]
    xv[:, :, :, 1, 0] = lv[:, :, :, 4:8]
    xv[:, :, :, 0, 1] = rv[:, :, :, 0:4]
    xv[:, :, :, 1, 1] = rv[:, :, :, 4:8]
    return [{"x": xin[i]} for i in range(n)]


def kernel(l_fmap, r_fmap, use_naive, max_disp):
    from concourse.bass_utils import run_bass_kernel_spmd

    l_fmap = np.asarray(l_fmap, dtype=np.float32)
    r_fmap = np.asarray(r_fmap, dtype=np.float32)
    assert int(max_disp) == D, f"kernel hardcoded for max_disp={D}"
    n, c, h, w = l_fmap.shape
    assert (n, c, h, w) == (N_CORES, C, H, W)

    nc = _get_nc()
    in_maps = _in_maps(l_fmap, r_fmap)
    res = run_bass_kernel_spmd(nc, in_maps, core_ids=list(range(N_CORES)))
    # decode: piece (b, wp, c) = [p', k, jbw(3), rr(32), ji(32)];
    # j' = 32*jbw + ji in [0, 96); element value =
    # cost[d = j'-p', h = 32b + rr, w = 256wp + 128k + 32c + j']
    arr = np.stack([np.asarray(res.results[i]["o"]).reshape(
        NB, 2, 4, 32, 2, 3, R, JI) for i in range(N_CORES)])
    # [n, b, wp, c, p', k, jbw, rr, ji] -> [.., k, p', jbw, ji, rr]
    arr = np.ascontiguousarray(arr.transpose(0, 1, 2, 3, 5, 4, 6, 8, 7))
    arr = arr.reshape(N_CORES, NB, 2, 4, 2, 32, 96, R)
    # arr dims [n(0), b(1), wp(2), c(3), k(4), p'(5), j'(6), rr(7)]
    s = arr.strides
    # diagonal view [n, b, wp, c, k, rr, p', d] with j' = p' + d
    v = np.lib.stride_tricks.as_strided(
        arr,
        shape=(N_CORES, NB, 2, 4, 2, R, 32, D),
        strides=(s[0], s[1], s[2], s[3], s[4], s[7], s[5] + s[6], s[6]),
    )
    # -> costv[n, d, h=(b,rr), vpos=(wp,k,c,p')];  w = vpos + d
    costv = v.transpose(0, 7, 1, 5, 2, 4, 3, 6).reshape(N_CORES, D, H, W)
    out = np.zeros((N_CORES, D, H, W), dtype=np.float32)
    for dd in range(D):
        out[:, dd, :, dd:] = costv[:, dd, :, :W - dd]
    return out


# revision 39
# speedup vs baseline: 1.0031x; 1.0031x over previous
"""CostVolume2D Trainium2 Bass kernel (v4: no scratch roundtrip).

cost[n,d,h,w] = mean_c l[n,c,h,w] * r[n,c,h,w-d]  (0 for w < d)
N=8, C=32, H=256, W=512, D=64.  Data-parallel over batch: core i handles n=i.

vs v2 (519 us): the DRAM scratch shear + gather + DVE block-transpose +
output store (50 MB of round-trip traffic, ~130K tiny 192B descriptors,
205 us of DVE transpose time) are gone.  The device stores the *band*
tiles in a windowed SBUF layout and the host extracts the diagonals with
a numpy as_strided view.  Per-core traffic: 16.8 MB in + 25.2 MB out,
80 DMAs, every descriptor >= 2 KB.

v3 lesson baked in here: engine copies with a strided dst run at ~5-6.7
cycles/elem (the cost model only counts ap size).  So the band layout is
[wp][jb(12)][rr(32)][ji(32)]: the psum->SBUF copy writes 64 B contiguous
runs (near-full rate), and the (c, k) store windows are whole jb blocks
-> 2 KB descriptors.

Device pipeline per 32-row block b (loads prefetched 2 blocks ahead on
sync, placed before the stores in stream order):
  - 2 half input DMAs: host-prebuilt [128, 8192] image (no l pad:
    the (wp1,k1) matmul is trimmed to 128 moving cols since w >= 512 is
    host-discarded), partition 32*jj+c holds row h = 32b + 4g + jj.
  - per row-quad, per wp: 8 matmuls into a 4-bank psum tile [128, 2048]
    (row q at 512-col offset; cols 192k+t hold M[v,w] with
    v = 256wp+128k+p, w = 256wp+128k+t; valid band t in [p, p+64)).
  - 1 copy per psum tile (scale 1/C, f32->bf16), alternating DVE/Act.
  - per (wp, c): 1 store DMA covering both k windows (band jb in
    [c+6k, +3), i.e. cols j in [192k+32c, +96)), 6 KB descriptors.
Host: as_strided gather j' = p' + d, transpose to [d, h, w], zero-fill
the w < d triangle.
"""

import numpy as np

_CACHE = {}

C, H, W, D = 32, 256, 512, 64
N_CORES = 8
R = 32                    # rows per block
NB = H // R               # 8 blocks
INCOLS = 8192             # [l g0-3 | r g0-3 | l g4-7 | r g4-7], 512 each
JI = 32                   # band inner-run elems (rr-major blocks of ji)
JB = 384 // JI            # 12 j-blocks per wp
BLK = R * JI              # 1024: elems per (partition, jb)
PIECE = 32 * 3 * BLK      # 98304 elems per (b, wp, c, k) piece


def _lcol(g):
    return (g // 4) * 4096 + (g % 4) * 512


def _rcol(g):
    return (g // 4) * 4096 + 2048 + (g % 4) * 512


def _build():
    import concourse.tile as tile
    from concourse import bacc, mybir
    from concourse.ap import AP

    f32 = mybir.dt.float32
    bf16 = mybir.dt.bfloat16

    nc = bacc.Bacc("TRN2", target_bir_lowering=False, debug=False)
    x_d = nc.dram_tensor("x", [NB, 128, INCOLS], bf16, kind="ExternalInput")
    o_d = nc.dram_tensor("o", [1, NB * 2 * 4 * 2 * PIECE], bf16,
                         kind="ExternalOutput")

    with tile.TileContext(nc) as tc:
        with (
            tc.tile_pool(name="io", bufs=2) as io_pool,
            tc.tile_pool(name="band", bufs=2) as band_pool,
            tc.tile_pool(name="psum", bufs=2, space="PSUM") as psum_pool,
        ):
            ci = 0
            in_tiles = {}

            # PE p-state warm-up: ~16 throwaway matmuls during the load
            # phase so the clock is ramped to max before real work.  The
            # psum tile is never read; operand values are irrelevant.
            wt = io_pool.tile([32, 512], bf16, tag="warm")
            nc.vector.memset(wt[:, :], 0)
            psw = psum_pool.tile([128, 2048], f32, tag="ps")
            for wi in range(16):
                nc.tensor.matmul(
                    psw[:, 192 * (wi % 8):192 * (wi % 8) + 192],
                    wt[:, 0:128], wt[:, 192:384],
                    start=True, stop=True, tile_position=(0, 0),
                    skip_group_check=True,
                )

            def load(bb):
                t = io_pool.tile([128, INCOLS], bf16, tag="in")
                # two half-loads (g 0-3 then g 4-7) so the first row-quads
                # can start before the whole block arrives
                for hh in range(2):
                    nc.sync.dma_start(
                        t[:, hh * 4096:(hh + 1) * 4096],
                        AP(x_d.ap().tensor,
                           bb * 128 * INCOLS + hh * 4096,
                           [[INCOLS, 128], [1, 4096]]))
                in_tiles[bb] = t

            load(0)
            load(1)
            for b in range(NB):
                it = in_tiles.pop(b)

                # band layout per partition:
                #   wp*(12*1024) + jb*1024 + rr*32 + ji   (j = 32*jb + ji)
                band = band_pool.tile([128, 2 * 12 * BLK], bf16, tag="band")
                bandv = band[:].rearrange(
                    "p (wp jb rr ji) -> p wp rr jb ji",
                    wp=2, jb=JB, rr=R, ji=JI)
                for rq in range(R // 4):
                    for wp in range(2):
                        ps = psum_pool.tile([128, 2048], f32, tag="ps")
                        # k-outer so consecutive matmuls hit different
                        # psum banks (q), avoiding same-bank turnaround
                        for k in range(2):
                            for q in range(4):
                                rr = rq * 4 + q
                                g = rr // 4
                                j = rr % 4
                                lrow = it[32 * j:32 * j + 32,
                                          _lcol(g):_lcol(g) + W]
                                rrow = it[32 * j:32 * j + 32,
                                          _rcol(g):_rcol(g) + W]
                                wb = (2 * wp + k) * 128
                                # (wp1,k1): cols w >= 512 are discarded by
                                # the host, so 128 moving cols suffice; the
                                # other blocks' col t=191 only maps to
                                # j'=95 > 94 = max(p'+d), also discarded
                                # (uncomputed psum = garbage in discarded
                                # slots only, NaN-emulation proven)
                                mc = 128 if wb == 384 else 191
                                nc.tensor.matmul(
                                    ps[:, 512 * q + 192 * k:
                                       512 * q + 192 * k + mc],
                                    rrow[:, wb:wb + 128],
                                    lrow[:, wb:wb + mc],
                                    start=True, stop=True,
                                    tile_position=(32 * j, 0),
                                )
                        # copy row-quad: src [p, q(4), jb(12), ji(32)] ->
                        # dst [p, rr(4): stride 32, jb: stride 1024, ji: 1]
                        src = ps[:].rearrange(
                            "p (r jb ji) -> p r jb ji",
                            r=4, jb=16, ji=JI)[:, :, 0:JB, :]
                        dst = bandv[:, wp, rq * 4:rq * 4 + 4, :, :]
                        if ci % 2 == 0:
                            nc.vector.tensor_scalar_mul(dst, src, 1.0 / C)
                        else:
                            nc.scalar.mul(dst, src, 1.0 / C)
                        ci += 1

                # prefetch the b+2 input before this block's stores so the
                # load isn't stuck behind store sem-waits in sync's stream
                if b + 2 < NB:
                    load(b + 2)

                # stores: piece (b, wp, c) = [p'(32), k(2), jbw(3), blk],
                # blk = rr*32 + ji; band jb = 6k + c + jbw.  One DMA per
                # (wp, c): both k windows (band k-stride 6*BLK is uniform).
                for wp in range(2):
                    for c in range(4):
                        pb = ((b * 2 + wp) * 4 + c) * 2 * PIECE
                        sv = band[32 * c:32 * c + 32, :].rearrange(
                            "p (wpp kk rest) -> p wpp kk rest",
                            wpp=2, kk=2, rest=6 * BLK)
                        src = sv[:, wp, :, c * BLK:c * BLK + 3 * BLK]
                        # drain-tail fix: the LAST block's stores split
                        # across sync+scalar (scalar's copies are all done
                        # by then, so no copy-blocking; halves the 8x1.1us
                        # issue serialization after the final copy)
                        eng = nc.scalar if (b == NB - 1 and c % 2 == 1) \
                            else nc.sync
                        eng.dma_start(
                            AP(o_d.ap().tensor, pb,
                               [[2 * 3 * BLK, 32], [3 * BLK, 2], [1, 3 * BLK]]),
                            src)
    nc.compile()
    return nc


def _get_nc():
    if "nc" not in _CACHE:
        _CACHE["nc"] = _build()
    return _CACHE["nc"]


def _in_maps(l_fmap, r_fmap):
    import ml_dtypes

    bf = ml_dtypes.bfloat16
    n = N_CORES
    # x[n, b, 32*jj + c, _lcol(g) + w] = l[n, c, 32b + 4g + jj, w]
    # x[n, b, 32*jj + c, _rcol(g) + w] = r[n, c, 32b + 4g + jj, w]
    xin = np.zeros((n, NB, 128, INCOLS), dtype=bf)
    # axes [n, c, b, g, jj, w] -> [n, b, (jj, c)=128, g, w]
    lv = l_fmap.astype(bf).reshape(n, C, NB, 8, 4, W).transpose(
        0, 2, 4, 1, 3, 5).reshape(n, NB, 128, 8, W)
    rv = r_fmap.astype(bf).reshape(n, C, NB, 8, 4, W).transpose(
        0, 2, 4, 1, 3, 5).reshape(n, NB, 128, 8, W)
    xv = xin.reshape(n, NB, 128, 2, 2, 4, W)  # [.., half, l/r, g%4, w]
    xv[:, :, :, 0, 0] = lv[:, :, :, 0:4]
    xv[:, :, :, 1, 0] = lv[:, :, :, 4:8]
    xv[:, :, :, 0, 1] = rv[:, :, :, 0:4]
    xv[:, :, :, 1, 1] = rv[:, :, :, 4:8]
    return [{"x": xin[i]} for i in range(n)]


def kernel(l_fmap, r_fmap, use_naive, max_disp):
    from concourse.bass_utils import run_bass_kernel_spmd

    l_fmap = np.asarray(l_fmap, dtype=np.float32)
    r_fmap = np.asarray(r_fmap, dtype=np.float32)
    assert int(max_disp) == D, f"kernel hardcoded for max_disp={D}"
    n, c, h, w = l_fmap.shape
    assert (n, c, h, w) == (N_CORES, C, H, W)

    nc = _get_nc()
    in_maps = _in_maps(l_fmap, r_fmap)
    res = run_bass_kernel_spmd(nc, in_maps, core_ids=list(range(N_CORES)))
    # decode: piece (b, wp, c) = [p', k, jbw(3), rr(32), ji(32)];
    # j' = 32*jbw + ji in [0, 96); element value =
    # cost[d = j'-p', h = 32b + rr, w = 256wp + 128k + 32c + j']
    arr = np.stack([np.asarray(res.results[i]["o"]).reshape(
        NB, 2, 4, 32, 2, 3, R, JI) for i in range(N_CORES)])
    # [n, b, wp, c, p', k, jbw, rr, ji] -> [.., k, p', jbw, ji, rr]
    arr = np.ascontiguousarray(arr.transpose(0, 1, 2, 3, 5, 4, 6, 8, 7))
    arr = arr.reshape(N_CORES, NB, 2, 4, 2, 32, 96, R)
    # arr dims [n(0), b(1), wp(2), c(3), k(4), p'(5), j'(6), rr(7)]
    s = arr.strides
    # diagonal view [n, b, wp, c, k, rr, p', d] with j' = p' + d
    v = np.lib.stride_tricks.as_strided(
        arr,
        shape=(N_CORES, NB, 2, 4, 2, R, 32, D),
        strides=(s[0], s[1], s[2], s[3], s[4], s[7], s[5] + s[6], s[6]),
    )
    # -> costv[n, d, h=(b,rr), vpos=(wp,k,c,p')];  w = vpos + d
    costv = v.transpose(0, 7, 1, 5, 2, 4, 3, 6).reshape(N_CORES, D, H, W)
    out = np.zeros((N_CORES, D, H, W), dtype=np.float32)
    for dd in range(D):
        out[:, dd, :, dd:] = costv[:, dd, :, :W - dd]
    return out


# revision 42
# speedup vs baseline: 1.1238x; 1.1204x over previous
"""CostVolume2D Trainium2 Bass kernel (v4: no scratch roundtrip).

cost[n,d,h,w] = mean_c l[n,c,h,w] * r[n,c,h,w-d]  (0 for w < d)
N=8, C=32, H=256, W=512, D=64.  Data-parallel over batch: core i handles n=i.

vs v2 (519 us): the DRAM scratch shear + gather + DVE block-transpose +
output store (50 MB of round-trip traffic, ~130K tiny 192B descriptors,
205 us of DVE transpose time) are gone.  The device stores the *band*
tiles in a windowed SBUF layout and the host extracts the diagonals with
a numpy as_strided view.  Per-core traffic: 16.8 MB in + 25.2 MB out,
80 DMAs, every descriptor >= 2 KB.

v3 lesson baked in here: engine copies with a strided dst run at ~5-6.7
cycles/elem (the cost model only counts ap size).  So the band layout is
[wp][jb(12)][rr(32)][ji(32)]: the psum->SBUF copy writes 64 B contiguous
runs (near-full rate), and the (c, k) store windows are whole jb blocks
-> 2 KB descriptors.

Device pipeline per 32-row block b (loads prefetched 2 blocks ahead on
sync, placed before the stores in stream order):
  - 2 half input DMAs: host-prebuilt [128, 8192] image (no l pad:
    the (wp1,k1) matmul is trimmed to 128 moving cols since w >= 512 is
    host-discarded), partition 32*jj+c holds row h = 32b + 4g + jj.
  - per row-quad, per wp: 8 matmuls into a 4-bank psum tile [128, 2048]
    (row q at 512-col offset; cols 192k+t hold M[v,w] with
    v = 256wp+128k+p, w = 256wp+128k+t; valid band t in [p, p+64)).
  - 1 copy per psum tile (scale 1/C, f32->bf16), alternating DVE/Act.
  - per (wp, c): 1 store DMA covering both k windows (band jb in
    [c+6k, +3), i.e. cols j in [192k+32c, +96)), 6 KB descriptors.
Host: as_strided gather j' = p' + d, transpose to [d, h, w], zero-fill
the w < d triangle.
"""

import numpy as np

_CACHE = {}

C, H, W, D = 32, 256, 512, 64
N_CORES = 8
R = 32                    # rows per block
NB = H // R               # 8 blocks
INCOLS = 8192             # [l g0-3 | r g0-3 | l g4-7 | r g4-7], 512 each
JI = 32                   # band inner-run elems (rr-major blocks of ji)
JB = 384 // JI            # 12 j-blocks per wp
BLK = R * JI              # 1024: elems per (partition, jb)
PIECE = 32 * 3 * BLK      # 98304 elems per (b, wp, c, k) piece


def _lcol(g):
    return (g // 4) * 4096 + (g % 4) * 512


def _rcol(g):
    return (g // 4) * 4096 + 2048 + (g % 4) * 512


def _build():
    import concourse.tile as tile
    from concourse import bacc, mybir
    from concourse.ap import AP

    f32 = mybir.dt.float32
    bf16 = mybir.dt.bfloat16

    nc = bacc.Bacc("TRN2", target_bir_lowering=False, debug=False)
    x_d = nc.dram_tensor("x", [NB, 128, INCOLS], bf16, kind="ExternalInput")
    o_d = nc.dram_tensor("o", [1, NB * 2 * 4 * 2 * PIECE], bf16,
                         kind="ExternalOutput")

    with tile.TileContext(nc) as tc:
        with (
            tc.tile_pool(name="io", bufs=2) as io_pool,
            tc.tile_pool(name="band", bufs=2) as band_pool,
            tc.tile_pool(name="psum", bufs=4, space="PSUM") as psum_pool,
        ):
            ci = 0
            in_tiles = {}

            # PE p-state warm-up: ~16 throwaway matmuls during the load
            # phase so the clock is ramped to max before real work.  The
            # psum tile is never read; operand values are irrelevant.
            wt = io_pool.tile([32, 512], bf16, tag="warm")
            nc.vector.memset(wt[:, :], 0)
            psw = psum_pool.tile([128, 1024], f32, tag="ps")
            for wi in range(16):
                nc.tensor.matmul(
                    psw[:, 192 * (wi % 4):192 * (wi % 4) + 192],
                    wt[:, 0:128], wt[:, 192:384],
                    start=True, stop=True, tile_position=(0, 0),
                    skip_group_check=True,
                )

            def load(bb):
                t = io_pool.tile([128, INCOLS], bf16, tag="in")
                # two half-loads (g 0-3 then g 4-7) so the first row-quads
                # can start before the whole block arrives
                for hh in range(2):
                    nc.sync.dma_start(
                        t[:, hh * 4096:(hh + 1) * 4096],
                        AP(x_d.ap().tensor,
                           bb * 128 * INCOLS + hh * 4096,
                           [[INCOLS, 128], [1, 4096]]))
                in_tiles[bb] = t

            load(0)
            load(1)
            for b in range(NB):
                it = in_tiles.pop(b)

                # band layout per partition:
                #   wp*(12*1024) + jb*1024 + rr*32 + ji   (j = 32*jb + ji)
                band = band_pool.tile([128, 2 * 12 * BLK], bf16, tag="band")
                bandv = band[:].rearrange(
                    "p (wp jb rr ji) -> p wp rr jb ji",
                    wp=2, jb=JB, rr=R, ji=JI)
                # pair-row psum tiles (2 banks), bufs=4: the PE stalls
                # ~1070ns at every tile boundary with a 2-deep ring (the
                # copy latency chain); a 4-deep ring hides it behind 3
                # tiles of matmul work.
                for rp in range(R // 2):
                    for wp in range(2):
                        ps = psum_pool.tile([128, 1024], f32, tag="ps")
                        # k-outer so consecutive matmuls hit different
                        # psum banks (r2), avoiding same-bank turnaround
                        for k in range(2):
                            for r2 in range(2):
                                rr = rp * 2 + r2
                                g = rr // 4
                                j = rr % 4
                                lrow = it[32 * j:32 * j + 32,
                                          _lcol(g):_lcol(g) + W]
                                rrow = it[32 * j:32 * j + 32,
                                          _rcol(g):_rcol(g) + W]
                                wb = (2 * wp + k) * 128
                                # (wp1,k1): cols w >= 512 are discarded by
                                # the host, so 128 moving cols suffice; the
                                # other blocks' col t=191 only maps to
                                # j'=95 > 94 = max(p'+d), also discarded
                                # (uncomputed psum = garbage in discarded
                                # slots only, NaN-emulation proven)
                                mc = 128 if wb == 384 else 191
                                nc.tensor.matmul(
                                    ps[:, 512 * r2 + 192 * k:
                                       512 * r2 + 192 * k + mc],
                                    rrow[:, wb:wb + 128],
                                    lrow[:, wb:wb + mc],
                                    start=True, stop=True,
                                    tile_position=(32 * j, 0),
                                )
                        # copy row-pair: src [p, r2(2), jb(12), ji(32)] ->
                        # dst [p, rr(2): stride 32, jb: stride 1024, ji: 1]
                        src = ps[:].rearrange(
                            "p (r jb ji) -> p r jb ji",
                            r=2, jb=16, ji=JI)[:, :, 0:JB, :]
                        dst = bandv[:, wp, rp * 2:rp * 2 + 2, :, :]
                        # Act is faster per copy (1.2 vs 0.96 GHz): give it
                        # 5 of every 9 to balance engine busy time
                        if ci % 9 < 4:
                            nc.vector.tensor_scalar_mul(dst, src, 1.0 / C)
                        else:
                            nc.scalar.mul(dst, src, 1.0 / C)
                        ci += 1

                # prefetch the b+2 input before this block's stores so the
                # load isn't stuck behind store sem-waits in sync's stream
                if b + 2 < NB:
                    load(b + 2)

                # stores: piece (b, wp, c) = [p'(32), k(2), jbw(3), blk],
                # blk = rr*32 + ji; band jb = 6k + c + jbw.  One DMA per
                # (wp, c): both k windows (band k-stride 6*BLK is uniform).
                for wp in range(2):
                    for c in range(4):
                        pb = ((b * 2 + wp) * 4 + c) * 2 * PIECE
                        sv = band[32 * c:32 * c + 32, :].rearrange(
                            "p (wpp kk rest) -> p wpp kk rest",
                            wpp=2, kk=2, rest=6 * BLK)
                        src = sv[:, wp, :, c * BLK:c * BLK + 3 * BLK]
                        # drain-tail fix: the LAST block's stores split
                        # across sync+scalar (scalar's copies are all done
                        # by then, so no copy-blocking; halves the 8x1.1us
                        # issue serialization after the final copy)
                        eng = nc.scalar if (b == NB - 1 and c % 2 == 1) \
                            else nc.sync
                        eng.dma_start(
                            AP(o_d.ap().tensor, pb,
                               [[2 * 3 * BLK, 32], [3 * BLK, 2], [1, 3 * BLK]]),
                            src)
    nc.compile()
    return nc


def _get_nc():
    if "nc" not in _CACHE:
        _CACHE["nc"] = _build()
    return _CACHE["nc"]


def _in_maps(l_fmap, r_fmap):
    import ml_dtypes

    bf = ml_dtypes.bfloat16
    n = N_CORES
    # x[n, b, 32*jj + c, _lcol(g) + w] = l[n, c, 32b + 4g + jj, w]
    # x[n, b, 32*jj + c, _rcol(g) + w] = r[n, c, 32b + 4g + jj, w]
    xin = np.zeros((n, NB, 128, INCOLS), dtype=bf)
    # axes [n, c, b, g, jj, w] -> [n, b, (jj, c)=128, g, w]
    lv = l_fmap.astype(bf).reshape(n, C, NB, 8, 4, W).transpose(
        0, 2, 4, 1, 3, 5).reshape(n, NB, 128, 8, W)
    rv = r_fmap.astype(bf).reshape(n, C, NB, 8, 4, W).transpose(
        0, 2, 4, 1, 3, 5).reshape(n, NB, 128, 8, W)
    xv = xin.reshape(n, NB, 128, 2, 2, 4, W)  # [.., half, l/r, g%4, w]
    xv[:, :, :, 0, 0] = lv[:, :, :, 0:4]
    xv[:, :, :, 1, 0] = lv[:, :, :, 4:8]
    xv[:, :, :, 0, 1] = rv[:, :, :, 0:4]
    xv[:, :, :, 1, 1] = rv[:, :, :, 4:8]
    return [{"x": xin[i]} for i in range(n)]


def kernel(l_fmap, r_fmap, use_naive, max_disp):
    from concourse.bass_utils import run_bass_kernel_spmd

    l_fmap = np.asarray(l_fmap, dtype=np.float32)
    r_fmap = np.asarray(r_fmap, dtype=np.float32)
    assert int(max_disp) == D, f"kernel hardcoded for max_disp={D}"
    n, c, h, w = l_fmap.shape
    assert (n, c, h, w) == (N_CORES, C, H, W)

    nc = _get_nc()
    in_maps = _in_maps(l_fmap, r_fmap)
    res = run_bass_kernel_spmd(nc, in_maps, core_ids=list(range(N_CORES)))
    # decode: piece (b, wp, c) = [p', k, jbw(3), rr(32), ji(32)];
    # j' = 32*jbw + ji in [0, 96); element value =
    # cost[d = j'-p', h = 32b + rr, w = 256wp + 128k + 32c + j']
    arr = np.stack([np.asarray(res.results[i]["o"]).reshape(
        NB, 2, 4, 32, 2, 3, R, JI) for i in range(N_CORES)])
    # [n, b, wp, c, p', k, jbw, rr, ji] -> [.., k, p', jbw, ji, rr]
    arr = np.ascontiguousarray(arr.transpose(0, 1, 2, 3, 5, 4, 6, 8, 7))
    arr = arr.reshape(N_CORES, NB, 2, 4, 2, 32, 96, R)
    # arr dims [n(0), b(1), wp(2), c(3), k(4), p'(5), j'(6), rr(7)]
    s = arr.strides
    # diagonal view [n, b, wp, c, k, rr, p', d] with j' = p' + d
    v = np.lib.stride_tricks.as_strided(
        arr,
        shape=(N_CORES, NB, 2, 4, 2, R, 32, D),
        strides=(s[0], s[1], s[2], s[3], s[4], s[7], s[5] + s[6], s[6]),
    )
    # -> costv[n, d, h=(b,rr), vpos=(wp,k,c,p')];  w = vpos + d
    costv = v.transpose(0, 7, 1, 5, 2, 4, 3, 6).reshape(N_CORES, D, H, W)
    out = np.zeros((N_CORES, D, H, W), dtype=np.float32)
    for dd in range(D):
        out[:, dd, :, dd:] = costv[:, dd, :, :W - dd]
    return out


# revision 46
# speedup vs baseline: 1.2027x; 1.0702x over previous
"""CostVolume2D Trainium2 Bass kernel (v4: no scratch roundtrip).

cost[n,d,h,w] = mean_c l[n,c,h,w] * r[n,c,h,w-d]  (0 for w < d)
N=8, C=32, H=256, W=512, D=64.  Data-parallel over batch: core i handles n=i.

vs v2 (519 us): the DRAM scratch shear + gather + DVE block-transpose +
output store (50 MB of round-trip traffic, ~130K tiny 192B descriptors,
205 us of DVE transpose time) are gone.  The device stores the *band*
tiles in a windowed SBUF layout and the host extracts the diagonals with
a numpy as_strided view.  Per-core traffic: 16.8 MB in + 25.2 MB out,
80 DMAs, every descriptor >= 2 KB.

v3 lesson baked in here: engine copies with a strided dst run at ~5-6.7
cycles/elem (the cost model only counts ap size).  So the band layout is
[wp][jb(12)][rr(32)][ji(32)]: the psum->SBUF copy writes 64 B contiguous
runs (near-full rate), and the (c, k) store windows are whole jb blocks
-> 2 KB descriptors.

Device pipeline per 32-row block b (loads prefetched 2 blocks ahead on
sync, placed before the stores in stream order):
  - 2 half input DMAs: host-prebuilt [128, 8192] image (no l pad:
    the (wp1,k1) matmul is trimmed to 128 moving cols since w >= 512 is
    host-discarded), partition 32*jj+c holds row h = 32b + 4g + jj.
  - per row-quad, per wp: 8 matmuls into a 4-bank psum tile [128, 2048]
    (row q at 512-col offset; cols 192k+t hold M[v,w] with
    v = 256wp+128k+p, w = 256wp+128k+t; valid band t in [p, p+64)).
  - 1 copy per psum tile (scale 1/C, f32->bf16), alternating DVE/Act.
  - per (wp, c): 1 store DMA covering both k windows (band jb in
    [c+6k, +3), i.e. cols j in [192k+32c, +96)), 6 KB descriptors.
Host: as_strided gather j' = p' + d, transpose to [d, h, w], zero-fill
the w < d triangle.
"""

import numpy as np

_CACHE = {}

C, H, W, D = 32, 256, 512, 64
N_CORES = 8
R = 32                    # rows per block
NB = H // R               # 8 blocks
INCOLS = 8192             # [l g0-3 | r g0-3 | l g4-7 | r g4-7], 512 each
JI = 32                   # band inner-run elems (rr-major blocks of ji)
JB = 384 // JI            # 12 j-blocks per wp
BLK = R * JI              # 1024: elems per (partition, jb)
PIECE = 32 * 3 * BLK      # 98304 elems per (b, wp, c, k) piece


def _lcol(g):
    return (g // 4) * 4096 + (g % 4) * 512


def _rcol(g):
    return (g // 4) * 4096 + 2048 + (g % 4) * 512


def _build():
    import concourse.tile as tile
    from concourse import bacc, mybir
    from concourse.ap import AP

    f32 = mybir.dt.float32
    bf16 = mybir.dt.bfloat16

    nc = bacc.Bacc("TRN2", target_bir_lowering=False, debug=False)
    x_d = nc.dram_tensor("x", [NB, 128, INCOLS], bf16, kind="ExternalInput")
    o_d = nc.dram_tensor("o", [1, NB * 2 * 4 * 2 * PIECE], bf16,
                         kind="ExternalOutput")

    with tile.TileContext(nc) as tc:
        with (
            tc.tile_pool(name="io", bufs=2) as io_pool,
            tc.tile_pool(name="band", bufs=2) as band_pool,
            tc.tile_pool(name="psum", bufs=4, space="PSUM") as psum_pool,
        ):
            ci = 0
            in_tiles = {}

            # PE p-state warm-up: ~16 throwaway matmuls during the load
            # phase so the clock is ramped to max before real work.  The
            # psum tile is never read; operand values are irrelevant.
            wt = io_pool.tile([32, 512], bf16, tag="warm")
            nc.vector.memset(wt[:, :], 0)
            psw = psum_pool.tile([128, 1024], f32, tag="ps")
            for wi in range(16):
                nc.tensor.matmul(
                    psw[:, 192 * (wi % 4):192 * (wi % 4) + 192],
                    wt[:, 0:128], wt[:, 192:384],
                    start=True, stop=True, tile_position=(0, 0),
                    skip_group_check=True,
                )

            def load(bb):
                t = io_pool.tile([128, INCOLS], bf16, tag="in")
                # two half-loads (g 0-3 then g 4-7) so the first row-quads
                # can start before the whole block arrives
                for hh in range(2):
                    nc.sync.dma_start(
                        t[:, hh * 4096:(hh + 1) * 4096],
                        AP(x_d.ap().tensor,
                           bb * 128 * INCOLS + hh * 4096,
                           [[INCOLS, 128], [1, 4096]]))
                in_tiles[bb] = t

            load(0)
            load(1)
            for b in range(NB):
                it = in_tiles.pop(b)

                # band layout per partition:
                #   wp*(12*1024) + jb*1024 + rr*32 + ji   (j = 32*jb + ji)
                band = band_pool.tile([128, 2 * 12 * BLK], bf16, tag="band")
                bandv = band[:].rearrange(
                    "p (wp jb rr ji) -> p wp rr jb ji",
                    wp=2, jb=JB, rr=R, ji=JI)
                # pair-row psum tiles (2 banks), bufs=4: the PE stalls
                # ~1070ns at every tile boundary with a 2-deep ring (the
                # copy latency chain); a 4-deep ring hides it behind 3
                # tiles of matmul work.
                for rp in range(R // 2):
                    for wp in range(2):
                        ps = psum_pool.tile([128, 1024], f32, tag="ps")
                        # k-outer so consecutive matmuls hit different
                        # psum banks (r2), avoiding same-bank turnaround
                        for k in range(2):
                            for r2 in range(2):
                                rr = rp * 2 + r2
                                g = rr // 4
                                j = rr % 4
                                lrow = it[32 * j:32 * j + 32,
                                          _lcol(g):_lcol(g) + W]
                                rrow = it[32 * j:32 * j + 32,
                                          _rcol(g):_rcol(g) + W]
                                wb = (2 * wp + k) * 128
                                # (wp1,k1): cols w >= 512 are discarded by
                                # the host, so 128 moving cols suffice; the
                                # other blocks' col t=191 only maps to
                                # j'=95 > 94 = max(p'+d), also discarded
                                # (uncomputed psum = garbage in discarded
                                # slots only, NaN-emulation proven)
                                mc = 128 if wb == 384 else 191
                                nc.tensor.matmul(
                                    ps[:, 512 * r2 + 192 * k:
                                       512 * r2 + 192 * k + mc],
                                    rrow[:, wb:wb + 128],
                                    lrow[:, wb:wb + mc],
                                    start=True, stop=True,
                                    tile_position=(32 * j, 0),
                                )
                        # copy row-pair: src [p, r2(2), jb(12), ji(32)] ->
                        # dst [p, rr(2): stride 32, jb: stride 1024, ji: 1]
                        # wp1: jb 10-11 (k1 cols t>=128) only feed
                        # host-discarded w>=512 slots -> skip copying
                        # (stale band garbage lands in discarded slots)
                        njb = 10 if wp == 1 else JB
                        src = ps[:].rearrange(
                            "p (r jb ji) -> p r jb ji",
                            r=2, jb=16, ji=JI)[:, :, 0:njb, :]
                        dst = bandv[:, wp, rp * 2:rp * 2 + 2, 0:njb, :]
                        # per-op cost ~equal on DVE/Act; (rp+wp) parity
                        # gives each engine half the cheap wp1 tiles
                        if (rp + wp) % 2 == 0:
                            nc.vector.tensor_scalar_mul(dst, src, 1.0 / C)
                        else:
                            nc.scalar.mul(dst, src, 1.0 / C)
                        ci += 1

                # prefetch the b+2 input before this block's stores so the
                # load isn't stuck behind store sem-waits in sync's stream
                if b + 2 < NB:
                    load(b + 2)

                # stores: piece (b, wp, c) = [p'(32), k(2), jbw(3), blk],
                # blk = rr*32 + ji; band jb = 6k + c + jbw.  One DMA per
                # (wp, c): both k windows (band k-stride 6*BLK is uniform).
                for wp in range(2):
                    for c in range(4):
                        pb = ((b * 2 + wp) * 4 + c) * 2 * PIECE
                        sv = band[32 * c:32 * c + 32, :].rearrange(
                            "p (wpp kk rest) -> p wpp kk rest",
                            wpp=2, kk=2, rest=6 * BLK)
                        src = sv[:, wp, :, c * BLK:c * BLK + 3 * BLK]
                        # drain-tail fix: the LAST block's stores split
                        # across sync+scalar (scalar's copies are all done
                        # by then, so no copy-blocking; halves the 8x1.1us
                        # issue serialization after the final copy)
                        eng = nc.scalar if (b == NB - 1 and c % 2 == 1) \
                            else nc.sync
                        eng.dma_start(
                            AP(o_d.ap().tensor, pb,
                               [[2 * 3 * BLK, 32], [3 * BLK, 2], [1, 3 * BLK]]),
                            src)
    nc.compile()
    return nc


def _get_nc():
    if "nc" not in _CACHE:
        _CACHE["nc"] = _build()
    return _CACHE["nc"]


def _in_maps(l_fmap, r_fmap):
    import ml_dtypes

    bf = ml_dtypes.bfloat16
    n = N_CORES
    # x[n, b, 32*jj + c, _lcol(g) + w] = l[n, c, 32b + 4g + jj, w]
    # x[n, b, 32*jj + c, _rcol(g) + w] = r[n, c, 32b + 4g + jj, w]
    xin = np.zeros((n, NB, 128, INCOLS), dtype=bf)
    # axes [n, c, b, g, jj, w] -> [n, b, (jj, c)=128, g, w]
    lv = l_fmap.astype(bf).reshape(n, C, NB, 8, 4, W).transpose(
        0, 2, 4, 1, 3, 5).reshape(n, NB, 128, 8, W)
    rv = r_fmap.astype(bf).reshape(n, C, NB, 8, 4, W).transpose(
        0, 2, 4, 1, 3, 5).reshape(n, NB, 128, 8, W)
    xv = xin.reshape(n, NB, 128, 2, 2, 4, W)  # [.., half, l/r, g%4, w]
    xv[:, :, :, 0, 0] = lv[:, :, :, 0:4]
    xv[:, :, :, 1, 0] = lv[:, :, :, 4:8]
    xv[:, :, :, 0, 1] = rv[:, :, :, 0:4]
    xv[:, :, :, 1, 1] = rv[:, :, :, 4:8]
    return [{"x": xin[i]} for i in range(n)]


def kernel(l_fmap, r_fmap, use_naive, max_disp):
    from concourse.bass_utils import run_bass_kernel_spmd

    l_fmap = np.asarray(l_fmap, dtype=np.float32)
    r_fmap = np.asarray(r_fmap, dtype=np.float32)
    assert int(max_disp) == D, f"kernel hardcoded for max_disp={D}"
    n, c, h, w = l_fmap.shape
    assert (n, c, h, w) == (N_CORES, C, H, W)

    nc = _get_nc()
    in_maps = _in_maps(l_fmap, r_fmap)
    res = run_bass_kernel_spmd(nc, in_maps, core_ids=list(range(N_CORES)))
    # decode: piece (b, wp, c) = [p', k, jbw(3), rr(32), ji(32)];
    # j' = 32*jbw + ji in [0, 96); element value =
    # cost[d = j'-p', h = 32b + rr, w = 256wp + 128k + 32c + j']
    arr = np.stack([np.asarray(res.results[i]["o"]).reshape(
        NB, 2, 4, 32, 2, 3, R, JI) for i in range(N_CORES)])
    # [n, b, wp, c, p', k, jbw, rr, ji] -> [.., k, p', jbw, ji, rr]
    arr = np.ascontiguousarray(arr.transpose(0, 1, 2, 3, 5, 4, 6, 8, 7))
    arr = arr.reshape(N_CORES, NB, 2, 4, 2, 32, 96, R)
    # arr dims [n(0), b(1), wp(2), c(3), k(4), p'(5), j'(6), rr(7)]
    s = arr.strides
    # diagonal view [n, b, wp, c, k, rr, p', d] with j' = p' + d
    v = np.lib.stride_tricks.as_strided(
        arr,
        shape=(N_CORES, NB, 2, 4, 2, R, 32, D),
        strides=(s[0], s[1], s[2], s[3], s[4], s[7], s[5] + s[6], s[6]),
    )
    # -> costv[n, d, h=(b,rr), vpos=(wp,k,c,p')];  w = vpos + d
    costv = v.transpose(0, 7, 1, 5, 2, 4, 3, 6).reshape(N_CORES, D, H, W)
    out = np.zeros((N_CORES, D, H, W), dtype=np.float32)
    for dd in range(D):
        out[:, dd, :, dd:] = costv[:, dd, :, :W - dd]
    return out


# revision 47
# speedup vs baseline: 1.2501x; 1.0393x over previous
"""CostVolume2D Trainium2 Bass kernel (v4: no scratch roundtrip).

cost[n,d,h,w] = mean_c l[n,c,h,w] * r[n,c,h,w-d]  (0 for w < d)
N=8, C=32, H=256, W=512, D=64.  Data-parallel over batch: core i handles n=i.

vs v2 (519 us): the DRAM scratch shear + gather + DVE block-transpose +
output store (50 MB of round-trip traffic, ~130K tiny 192B descriptors,
205 us of DVE transpose time) are gone.  The device stores the *band*
tiles in a windowed SBUF layout and the host extracts the diagonals with
a numpy as_strided view.  Per-core traffic: 16.8 MB in + 25.2 MB out,
80 DMAs, every descriptor >= 2 KB.

v3 lesson baked in here: engine copies with a strided dst run at ~5-6.7
cycles/elem (the cost model only counts ap size).  So the band layout is
[wp][jb(12)][rr(32)][ji(32)]: the psum->SBUF copy writes 64 B contiguous
runs (near-full rate), and the (c, k) store windows are whole jb blocks
-> 2 KB descriptors.

Device pipeline per 32-row block b (loads prefetched 2 blocks ahead on
sync, placed before the stores in stream order):
  - 2 half input DMAs: host-prebuilt [128, 8192] image (no l pad:
    the (wp1,k1) matmul is trimmed to 128 moving cols since w >= 512 is
    host-discarded), partition 32*jj+c holds row h = 32b + 4g + jj.
  - per row-quad, per wp: 8 matmuls into a 4-bank psum tile [128, 2048]
    (row q at 512-col offset; cols 192k+t hold M[v,w] with
    v = 256wp+128k+p, w = 256wp+128k+t; valid band t in [p, p+64)).
  - 1 copy per psum tile (scale 1/C, f32->bf16), alternating DVE/Act.
  - per (wp, c): 1 store DMA covering both k windows (band jb in
    [c+6k, +3), i.e. cols j in [192k+32c, +96)), 6 KB descriptors.
Host: as_strided gather j' = p' + d, transpose to [d, h, w], zero-fill
the w < d triangle.
"""

import numpy as np

_CACHE = {}

C, H, W, D = 32, 256, 512, 64
N_CORES = 8
R = 32                    # rows per block
NB = H // R               # 8 blocks
INCOLS = 8192             # [l g0-3 | r g0-3 | l g4-7 | r g4-7], 512 each
JI = 32                   # band inner-run elems (rr-major blocks of ji)
JB = 384 // JI            # 12 j-blocks per wp
BLK = R * JI              # 1024: elems per (partition, jb)
PIECE = 32 * 3 * BLK      # 98304 elems per (b, wp, c, k) piece


def _lcol(g):
    return (g // 4) * 4096 + (g % 4) * 512


def _rcol(g):
    return (g // 4) * 4096 + 2048 + (g % 4) * 512


def _build():
    import concourse.tile as tile
    from concourse import bacc, mybir
    from concourse.ap import AP

    f32 = mybir.dt.float32
    bf16 = mybir.dt.bfloat16

    nc = bacc.Bacc("TRN2", target_bir_lowering=False, debug=False)
    x_d = nc.dram_tensor("x", [NB, 128, INCOLS], bf16, kind="ExternalInput")
    o_d = nc.dram_tensor("o", [1, NB * 2 * 4 * 2 * PIECE], bf16,
                         kind="ExternalOutput")

    with tile.TileContext(nc) as tc:
        with (
            tc.tile_pool(name="io", bufs=3) as io_pool,
            tc.tile_pool(name="band", bufs=2) as band_pool,
            tc.tile_pool(name="psum", bufs=4, space="PSUM") as psum_pool,
        ):
            ci = 0
            in_tiles = {}

            # PE p-state warm-up: ~16 throwaway matmuls during the load
            # phase so the clock is ramped to max before real work.  The
            # psum tile is never read; operand values are irrelevant.
            wt = io_pool.tile([32, 512], bf16, tag="warm")
            nc.vector.memset(wt[:, :], 0)
            psw = psum_pool.tile([128, 1024], f32, tag="ps")
            for wi in range(16):
                nc.tensor.matmul(
                    psw[:, 192 * (wi % 4):192 * (wi % 4) + 192],
                    wt[:, 0:128], wt[:, 192:384],
                    start=True, stop=True, tile_position=(0, 0),
                    skip_group_check=True,
                )

            def load(bb):
                t = io_pool.tile([128, INCOLS], bf16, tag="in")
                # two half-loads (g 0-3 then g 4-7) so the first row-quads
                # can start before the whole block arrives
                for hh in range(2):
                    nc.sync.dma_start(
                        t[:, hh * 4096:(hh + 1) * 4096],
                        AP(x_d.ap().tensor,
                           bb * 128 * INCOLS + hh * 4096,
                           [[INCOLS, 128], [1, 4096]]))
                in_tiles[bb] = t

            load(0)
            load(1)
            for b in range(NB):
                it = in_tiles.pop(b)

                # band layout per partition:
                #   wp*(12*1024) + jb*1024 + rr*32 + ji   (j = 32*jb + ji)
                band = band_pool.tile([128, 2 * 12 * BLK], bf16, tag="band")
                bandv = band[:].rearrange(
                    "p (wp jb rr ji) -> p wp rr jb ji",
                    wp=2, jb=JB, rr=R, ji=JI)
                # pair-row psum tiles (2 banks), bufs=4: the PE stalls
                # ~1070ns at every tile boundary with a 2-deep ring (the
                # copy latency chain); a 4-deep ring hides it behind 3
                # tiles of matmul work.
                for rp in range(R // 2):
                    for wp in range(2):
                        ps = psum_pool.tile([128, 1024], f32, tag="ps")
                        # k-outer so consecutive matmuls hit different
                        # psum banks (r2), avoiding same-bank turnaround
                        for k in range(2):
                            for r2 in range(2):
                                rr = rp * 2 + r2
                                g = rr // 4
                                j = rr % 4
                                lrow = it[32 * j:32 * j + 32,
                                          _lcol(g):_lcol(g) + W]
                                rrow = it[32 * j:32 * j + 32,
                                          _rcol(g):_rcol(g) + W]
                                wb = (2 * wp + k) * 128
                                # (wp1,k1): cols w >= 512 are discarded by
                                # the host, so 128 moving cols suffice; the
                                # other blocks' col t=191 only maps to
                                # j'=95 > 94 = max(p'+d), also discarded
                                # (uncomputed psum = garbage in discarded
                                # slots only, NaN-emulation proven)
                                mc = 128 if wb == 384 else 191
                                nc.tensor.matmul(
                                    ps[:, 512 * r2 + 192 * k:
                                       512 * r2 + 192 * k + mc],
                                    rrow[:, wb:wb + 128],
                                    lrow[:, wb:wb + mc],
                                    start=True, stop=True,
                                    tile_position=(32 * j, 0),
                                )
                        # copy row-pair: src [p, r2(2), jb(12), ji(32)] ->
                        # dst [p, rr(2): stride 32, jb: stride 1024, ji: 1]
                        # wp1: jb 10-11 (k1 cols t>=128) only feed
                        # host-discarded w>=512 slots -> skip copying
                        # (stale band garbage lands in discarded slots)
                        njb = 10 if wp == 1 else JB
                        src = ps[:].rearrange(
                            "p (r jb ji) -> p r jb ji",
                            r=2, jb=16, ji=JI)[:, :, 0:njb, :]
                        dst = bandv[:, wp, rp * 2:rp * 2 + 2, 0:njb, :]
                        # per-op cost ~equal on DVE/Act; (rp+wp) parity
                        # gives each engine half the cheap wp1 tiles
                        if (rp + wp) % 2 == 0:
                            nc.vector.tensor_scalar_mul(dst, src, 1.0 / C)
                        else:
                            nc.scalar.mul(dst, src, 1.0 / C)
                        ci += 1

                # prefetch the b+2 input before this block's stores so the
                # load isn't stuck behind store sem-waits in sync's stream
                if b + 2 < NB:
                    load(b + 2)

                # stores: piece (b, wp, c) = [p'(32), k(2), jbw(3), blk],
                # blk = rr*32 + ji; band jb = 6k + c + jbw.  One DMA per
                # (wp, c): both k windows (band k-stride 6*BLK is uniform).
                for wp in range(2):
                    for c in range(4):
                        pb = ((b * 2 + wp) * 4 + c) * 2 * PIECE
                        sv = band[32 * c:32 * c + 32, :].rearrange(
                            "p (wpp kk rest) -> p wpp kk rest",
                            wpp=2, kk=2, rest=6 * BLK)
                        src = sv[:, wp, :, c * BLK:c * BLK + 3 * BLK]
                        # drain-tail fix: the LAST block's stores split
                        # across sync+scalar (scalar's copies are all done
                        # by then, so no copy-blocking; halves the 8x1.1us
                        # issue serialization after the final copy)
                        eng = nc.scalar if (b == NB - 1 and c % 2 == 1) \
                            else nc.sync
                        eng.dma_start(
                            AP(o_d.ap().tensor, pb,
                               [[2 * 3 * BLK, 32], [3 * BLK, 2], [1, 3 * BLK]]),
                            src)
    nc.compile()
    return nc


def _get_nc():
    if "nc" not in _CACHE:
        _CACHE["nc"] = _build()
    return _CACHE["nc"]


def _in_maps(l_fmap, r_fmap):
    import ml_dtypes

    bf = ml_dtypes.bfloat16
    n = N_CORES
    # x[n, b, 32*jj + c, _lcol(g) + w] = l[n, c, 32b + 4g + jj, w]
    # x[n, b, 32*jj + c, _rcol(g) + w] = r[n, c, 32b + 4g + jj, w]
    xin = np.zeros((n, NB, 128, INCOLS), dtype=bf)
    # axes [n, c, b, g, jj, w] -> [n, b, (jj, c)=128, g, w]
    lv = l_fmap.astype(bf).reshape(n, C, NB, 8, 4, W).transpose(
        0, 2, 4, 1, 3, 5).reshape(n, NB, 128, 8, W)
    rv = r_fmap.astype(bf).reshape(n, C, NB, 8, 4, W).transpose(
        0, 2, 4, 1, 3, 5).reshape(n, NB, 128, 8, W)
    xv = xin.reshape(n, NB, 128, 2, 2, 4, W)  # [.., half, l/r, g%4, w]
    xv[:, :, :, 0, 0] = lv[:, :, :, 0:4]
    xv[:, :, :, 1, 0] = lv[:, :, :, 4:8]
    xv[:, :, :, 0, 1] = rv[:, :, :, 0:4]
    xv[:, :, :, 1, 1] = rv[:, :, :, 4:8]
    return [{"x": xin[i]} for i in range(n)]


def kernel(l_fmap, r_fmap, use_naive, max_disp):
    from concourse.bass_utils import run_bass_kernel_spmd

    l_fmap = np.asarray(l_fmap, dtype=np.float32)
    r_fmap = np.asarray(r_fmap, dtype=np.float32)
    assert int(max_disp) == D, f"kernel hardcoded for max_disp={D}"
    n, c, h, w = l_fmap.shape
    assert (n, c, h, w) == (N_CORES, C, H, W)

    nc = _get_nc()
    in_maps = _in_maps(l_fmap, r_fmap)
    res = run_bass_kernel_spmd(nc, in_maps, core_ids=list(range(N_CORES)))
    # decode: piece (b, wp, c) = [p', k, jbw(3), rr(32), ji(32)];
    # j' = 32*jbw + ji in [0, 96); element value =
    # cost[d = j'-p', h = 32b + rr, w = 256wp + 128k + 32c + j']
    arr = np.stack([np.asarray(res.results[i]["o"]).reshape(
        NB, 2, 4, 32, 2, 3, R, JI) for i in range(N_CORES)])
    # [n, b, wp, c, p', k, jbw, rr, ji] -> [.., k, p', jbw, ji, rr]
    arr = np.ascontiguousarray(arr.transpose(0, 1, 2, 3, 5, 4, 6, 8, 7))
    arr = arr.reshape(N_CORES, NB, 2, 4, 2, 32, 96, R)
    # arr dims [n(0), b(1), wp(2), c(3), k(4), p'(5), j'(6), rr(7)]
    s = arr.strides
    # diagonal view [n, b, wp, c, k, rr, p', d] with j' = p' + d
    v = np.lib.stride_tricks.as_strided(
        arr,
        shape=(N_CORES, NB, 2, 4, 2, R, 32, D),
        strides=(s[0], s[1], s[2], s[3], s[4], s[7], s[5] + s[6], s[6]),
    )
    # -> costv[n, d, h=(b,rr), vpos=(wp,k,c,p')];  w = vpos + d
    costv = v.transpose(0, 7, 1, 5, 2, 4, 3, 6).reshape(N_CORES, D, H, W)
    out = np.zeros((N_CORES, D, H, W), dtype=np.float32)
    for dd in range(D):
        out[:, dd, :, dd:] = costv[:, dd, :, :W - dd]
    return out


# revision 48
# speedup vs baseline: 1.2542x; 1.0033x over previous
"""CostVolume2D Trainium2 Bass kernel (v4: no scratch roundtrip).

cost[n,d,h,w] = mean_c l[n,c,h,w] * r[n,c,h,w-d]  (0 for w < d)
N=8, C=32, H=256, W=512, D=64.  Data-parallel over batch: core i handles n=i.

vs v2 (519 us): the DRAM scratch shear + gather + DVE block-transpose +
output store (50 MB of round-trip traffic, ~130K tiny 192B descriptors,
205 us of DVE transpose time) are gone.  The device stores the *band*
tiles in a windowed SBUF layout and the host extracts the diagonals with
a numpy as_strided view.  Per-core traffic: 16.8 MB in + 25.2 MB out,
80 DMAs, every descriptor >= 2 KB.

v3 lesson baked in here: engine copies with a strided dst run at ~5-6.7
cycles/elem (the cost model only counts ap size).  So the band layout is
[wp][jb(12)][rr(32)][ji(32)]: the psum->SBUF copy writes 64 B contiguous
runs (near-full rate), and the (c, k) store windows are whole jb blocks
-> 2 KB descriptors.

Device pipeline per 32-row block b (loads prefetched 2 blocks ahead on
sync, placed before the stores in stream order):
  - 2 half input DMAs: host-prebuilt [128, 8192] image (no l pad:
    the (wp1,k1) matmul is trimmed to 128 moving cols since w >= 512 is
    host-discarded), partition 32*jj+c holds row h = 32b + 4g + jj.
  - per row-quad, per wp: 8 matmuls into a 4-bank psum tile [128, 2048]
    (row q at 512-col offset; cols 192k+t hold M[v,w] with
    v = 256wp+128k+p, w = 256wp+128k+t; valid band t in [p, p+64)).
  - 1 copy per psum tile (scale 1/C, f32->bf16), alternating DVE/Act.
  - per (wp, c): 1 store DMA covering both k windows (band jb in
    [c+6k, +3), i.e. cols j in [192k+32c, +96)), 6 KB descriptors.
Host: as_strided gather j' = p' + d, transpose to [d, h, w], zero-fill
the w < d triangle.
"""

import numpy as np

_CACHE = {}

C, H, W, D = 32, 256, 512, 64
N_CORES = 8
R = 32                    # rows per block
NB = H // R               # 8 blocks
INCOLS = 8192             # [l g0-3 | r g0-3 | l g4-7 | r g4-7], 512 each
JI = 32                   # band inner-run elems (rr-major blocks of ji)
JB = 384 // JI            # 12 j-blocks per wp
BLK = R * JI              # 1024: elems per (partition, jb)
PIECE = 32 * 3 * BLK      # 98304 elems per (b, wp, c, k) piece


def _lcol(g):
    return (g // 4) * 4096 + (g % 4) * 512


def _rcol(g):
    return (g // 4) * 4096 + 2048 + (g % 4) * 512


def _build():
    import concourse.tile as tile
    from concourse import bacc, mybir
    from concourse.ap import AP

    f32 = mybir.dt.float32
    bf16 = mybir.dt.bfloat16

    nc = bacc.Bacc("TRN2", target_bir_lowering=False, debug=False)
    x_d = nc.dram_tensor("x", [NB, 128, INCOLS], bf16, kind="ExternalInput")
    o_d = nc.dram_tensor("o", [1, NB * 2 * 4 * 2 * PIECE], bf16,
                         kind="ExternalOutput")

    with tile.TileContext(nc) as tc:
        with (
            tc.tile_pool(name="io", bufs=3) as io_pool,
            tc.tile_pool(name="band", bufs=3) as band_pool,
            tc.tile_pool(name="psum", bufs=4, space="PSUM") as psum_pool,
        ):
            ci = 0
            in_tiles = {}

            # PE p-state warm-up: ~16 throwaway matmuls during the load
            # phase so the clock is ramped to max before real work.  The
            # psum tile is never read; operand values are irrelevant.
            wt = io_pool.tile([32, 512], bf16, tag="warm")
            nc.vector.memset(wt[:, :], 0)
            psw = psum_pool.tile([128, 1024], f32, tag="ps")
            for wi in range(16):
                nc.tensor.matmul(
                    psw[:, 192 * (wi % 4):192 * (wi % 4) + 192],
                    wt[:, 0:128], wt[:, 192:384],
                    start=True, stop=True, tile_position=(0, 0),
                    skip_group_check=True,
                )

            def load(bb):
                t = io_pool.tile([128, INCOLS], bf16, tag="in")
                # two half-loads (g 0-3 then g 4-7) so the first row-quads
                # can start before the whole block arrives
                for hh in range(2):
                    nc.sync.dma_start(
                        t[:, hh * 4096:(hh + 1) * 4096],
                        AP(x_d.ap().tensor,
                           bb * 128 * INCOLS + hh * 4096,
                           [[INCOLS, 128], [1, 4096]]))
                in_tiles[bb] = t

            load(0)
            load(1)
            for b in range(NB):
                it = in_tiles.pop(b)

                # band layout per partition:
                #   wp*(12*1024) + jb*1024 + rr*32 + ji   (j = 32*jb + ji)
                band = band_pool.tile([128, 2 * 12 * BLK], bf16, tag="band")
                bandv = band[:].rearrange(
                    "p (wp jb rr ji) -> p wp rr jb ji",
                    wp=2, jb=JB, rr=R, ji=JI)
                # pair-row psum tiles (2 banks), bufs=4: the PE stalls
                # ~1070ns at every tile boundary with a 2-deep ring (the
                # copy latency chain); a 4-deep ring hides it behind 3
                # tiles of matmul work.
                for rp in range(R // 2):
                    for wp in range(2):
                        ps = psum_pool.tile([128, 1024], f32, tag="ps")
                        # k-outer so consecutive matmuls hit different
                        # psum banks (r2), avoiding same-bank turnaround
                        for k in range(2):
                            for r2 in range(2):
                                rr = rp * 2 + r2
                                g = rr // 4
                                j = rr % 4
                                lrow = it[32 * j:32 * j + 32,
                                          _lcol(g):_lcol(g) + W]
                                rrow = it[32 * j:32 * j + 32,
                                          _rcol(g):_rcol(g) + W]
                                wb = (2 * wp + k) * 128
                                # (wp1,k1): cols w >= 512 are discarded by
                                # the host, so 128 moving cols suffice; the
                                # other blocks' col t=191 only maps to
                                # j'=95 > 94 = max(p'+d), also discarded
                                # (uncomputed psum = garbage in discarded
                                # slots only, NaN-emulation proven)
                                mc = 128 if wb == 384 else 191
                                nc.tensor.matmul(
                                    ps[:, 512 * r2 + 192 * k:
                                       512 * r2 + 192 * k + mc],
                                    rrow[:, wb:wb + 128],
                                    lrow[:, wb:wb + mc],
                                    start=True, stop=True,
                                    tile_position=(32 * j, 0),
                                )
                        # copy row-pair: src [p, r2(2), jb(12), ji(32)] ->
                        # dst [p, rr(2): stride 32, jb: stride 1024, ji: 1]
                        # wp1: jb 10-11 (k1 cols t>=128) only feed
                        # host-discarded w>=512 slots -> skip copying
                        # (stale band garbage lands in discarded slots)
                        njb = 10 if wp == 1 else JB
                        src = ps[:].rearrange(
                            "p (r jb ji) -> p r jb ji",
                            r=2, jb=16, ji=JI)[:, :, 0:njb, :]
                        dst = bandv[:, wp, rp * 2:rp * 2 + 2, 0:njb, :]
                        # per-op cost ~equal on DVE/Act; (rp+wp) parity
                        # gives each engine half the cheap wp1 tiles
                        if (rp + wp) % 2 == 0:
                            nc.vector.tensor_scalar_mul(dst, src, 1.0 / C)
                        else:
                            nc.scalar.mul(dst, src, 1.0 / C)
                        ci += 1

                # prefetch the b+2 input before this block's stores so the
                # load isn't stuck behind store sem-waits in sync's stream
                if b + 2 < NB:
                    load(b + 2)

                # stores: piece (b, wp, c) = [p'(32), k(2), jbw(3), blk],
                # blk = rr*32 + ji; band jb = 6k + c + jbw.  One DMA per
                # (wp, c): both k windows (band k-stride 6*BLK is uniform).
                for wp in range(2):
                    for c in range(4):
                        pb = ((b * 2 + wp) * 4 + c) * 2 * PIECE
                        sv = band[32 * c:32 * c + 32, :].rearrange(
                            "p (wpp kk rest) -> p wpp kk rest",
                            wpp=2, kk=2, rest=6 * BLK)
                        src = sv[:, wp, :, c * BLK:c * BLK + 3 * BLK]
                        # drain-tail fix: the LAST block's stores split
                        # across sync+scalar (scalar's copies are all done
                        # by then, so no copy-blocking; halves the 8x1.1us
                        # issue serialization after the final copy)
                        eng = nc.scalar if (b == NB - 1 and c % 2 == 1) \
                            else nc.sync
                        eng.dma_start(
                            AP(o_d.ap().tensor, pb,
                               [[2 * 3 * BLK, 32], [3 * BLK, 2], [1, 3 * BLK]]),
                            src)
    nc.compile()
    return nc


def _get_nc():
    if "nc" not in _CACHE:
        _CACHE["nc"] = _build()
    return _CACHE["nc"]


def _in_maps(l_fmap, r_fmap):
    import ml_dtypes

    bf = ml_dtypes.bfloat16
    n = N_CORES
    # x[n, b, 32*jj + c, _lcol(g) + w] = l[n, c, 32b + 4g + jj, w]
    # x[n, b, 32*jj + c, _rcol(g) + w] = r[n, c, 32b + 4g + jj, w]
    xin = np.zeros((n, NB, 128, INCOLS), dtype=bf)
    # axes [n, c, b, g, jj, w] -> [n, b, (jj, c)=128, g, w]
    lv = l_fmap.astype(bf).reshape(n, C, NB, 8, 4, W).transpose(
        0, 2, 4, 1, 3, 5).reshape(n, NB, 128, 8, W)
    rv = r_fmap.astype(bf).reshape(n, C, NB, 8, 4, W).transpose(
        0, 2, 4, 1, 3, 5).reshape(n, NB, 128, 8, W)
    xv = xin.reshape(n, NB, 128, 2, 2, 4, W)  # [.., half, l/r, g%4, w]
    xv[:, :, :, 0, 0] = lv[:, :, :, 0:4]
    xv[:, :, :, 1, 0] = lv[:, :, :, 4:8]
    xv[:, :, :, 0, 1] = rv[:, :, :, 0:4]
    xv[:, :, :, 1, 1] = rv[:, :, :, 4:8]
    return [{"x": xin[i]} for i in range(n)]


def kernel(l_fmap, r_fmap, use_naive, max_disp):
    from concourse.bass_utils import run_bass_kernel_spmd

    l_fmap = np.asarray(l_fmap, dtype=np.float32)
    r_fmap = np.asarray(r_fmap, dtype=np.float32)
    assert int(max_disp) == D, f"kernel hardcoded for max_disp={D}"
    n, c, h, w = l_fmap.shape
    assert (n, c, h, w) == (N_CORES, C, H, W)

    nc = _get_nc()
    in_maps = _in_maps(l_fmap, r_fmap)
    res = run_bass_kernel_spmd(nc, in_maps, core_ids=list(range(N_CORES)))
    # decode: piece (b, wp, c) = [p', k, jbw(3), rr(32), ji(32)];
    # j' = 32*jbw + ji in [0, 96); element value =
    # cost[d = j'-p', h = 32b + rr, w = 256wp + 128k + 32c + j']
    arr = np.stack([np.asarray(res.results[i]["o"]).reshape(
        NB, 2, 4, 32, 2, 3, R, JI) for i in range(N_CORES)])
    # [n, b, wp, c, p', k, jbw, rr, ji] -> [.., k, p', jbw, ji, rr]
    arr = np.ascontiguousarray(arr.transpose(0, 1, 2, 3, 5, 4, 6, 8, 7))
    arr = arr.reshape(N_CORES, NB, 2, 4, 2, 32, 96, R)
    # arr dims [n(0), b(1), wp(2), c(3), k(4), p'(5), j'(6), rr(7)]
    s = arr.strides
    # diagonal view [n, b, wp, c, k, rr, p', d] with j' = p' + d
    v = np.lib.stride_tricks.as_strided(
        arr,
        shape=(N_CORES, NB, 2, 4, 2, R, 32, D),
        strides=(s[0], s[1], s[2], s[3], s[4], s[7], s[5] + s[6], s[6]),
    )
    # -> costv[n, d, h=(b,rr), vpos=(wp,k,c,p')];  w = vpos + d
    costv = v.transpose(0, 7, 1, 5, 2, 4, 3, 6).reshape(N_CORES, D, H, W)
    out = np.zeros((N_CORES, D, H, W), dtype=np.float32)
    for dd in range(D):
        out[:, dd, :, dd:] = costv[:, dd, :, :W - dd]
    return out
